# revision 1
# baseline (speedup 1.0000x reference)
"""Mex pooling kernel for Trainium2 (8 NeuronCores, data-parallel over batch).

Problem: y[b,i,oh,ow] = logsumexp_k(P[b,oh,ow,:] + O[i,:]) - log(K)
  with P = 3x3/stride-2/pad-1 patches over (C=64,H=64,W=64), K = 576, NI = 256.

Math trick: no max-shift needed (values are small), and
  exp(P + O) patch GEMM == (implicit-im2col) sum over 9 taps of
  exp(x)[c, 2oh-1+fh, 2ow-1+fw] @ exp(O)[c,fh,fw; i],
with exp(0)=1 at the zero-padding. So:
  - exp(x) is computed once into 4 (h,w)-parity planes with a 1-wide halo of
    ones at h=-1 / w=-1 (the only padded edges reachable with stride 2),
  - each tap is a contiguous block read of one parity plane (no im2col copies),
  - taps accumulate in PSUM over 9 matmuls; two samples run concurrently in
    the PE array via row-group packing (tile_position 0/64, K=64 each),
  - y = Ln(PSUM * 1/K) on the scalar engine, DMA'd straight to the output.

Matmuls run in float32r (TF32-like, ~1e-4 rel err) at full PE rate.

Variants:
  v1: per-pair exp -> GEMM -> Ln streaming (ACT table switches per pair)
  v2: all exps up front (planes for all pairs persist), then GEMM+Ln per pair
"""
import sys

sys.path.insert(0, "/opt/trn_rl_repo")

import numpy as np

N_CORES = 8
B, C, H, W = 64, 64, 64, 64
NI = 256
KTOT = 576
OH = OW = 32
B_CORE = B // N_CORES          # 8 samples per core
N_PAIRS = B_CORE // 2          # processed 2 samples at a time

PLANE_HW = {(0, 0): (33, 33), (0, 1): (33, 32), (1, 0): (32, 33), (1, 1): (32, 32)}

_compiled = None


def build_nc(reps: int = 1, variant: str = "v2"):
    import concourse.bacc as bacc
    import concourse.mybir as mybir
    from concourse import tile

    F32 = mybir.dt.float32
    F32R = mybir.dt.float32r
    Exp = mybir.ActivationFunctionType.Exp
    Ln = mybir.ActivationFunctionType.Ln

    nc = bacc.Bacc("TRN2", target_bir_lowering=False, debug=False,
                   num_devices=N_CORES)
    x_d = nc.dram_tensor("x", [B_CORE, C, H, W], F32, kind="ExternalInput").ap()
    o_d = nc.dram_tensor("offs", [64, 9 * 2 * 128], F32, kind="ExternalInput").ap()
    y_d = nc.dram_tensor("y", [B_CORE, NI, OH, OW], F32, kind="ExternalOutput").ap()
    xf = x_d.rearrange("s c h w -> (s c) (h w)")

    with tile.TileContext(nc) as tc:
        with tc.tile_pool(name="const", bufs=1) as cpool, \
             tc.tile_pool(name="xp", bufs=2 if variant == "v1" else 3) as xpool, \
             tc.tile_pool(name="planes", bufs=2 if variant == "v1" else 1) as ppool, \
             tc.tile_pool(name="psum", bufs=1 if variant == "v4" else 2, space="PSUM") as pspool, \
             tc.tile_pool(name="outp", bufs=4) as opool:
            # exp(offsets), duplicated on both row-groups:
            # [128=(rowgroup, c), (t, ih, m)]
            ostage = cpool.tile([128, 2304], F32, tag="ostage")
            nc.sync.dma_start(ostage[0:64, :], o_d[:, :])
            nc.sync.dma_start(ostage[64:128, :], o_d[:, :])
            expO = cpool.tile([128, 2304], F32R, tag="expO")
            nc.scalar.activation(expO[:], ostage[:], Exp)
            expO4 = expO.rearrange("p (t i m) -> p t i m", t=9, i=2)

            def load_and_exp(pair, uid, plane_tag_suffix, no_exp=False):
                x_t = xpool.tile([128, H * W], F32, tag="x", name=f"x_{uid}")
                nc.sync.dma_start(x_t[:], xf[pair * 128:(pair + 1) * 128, :])
                x_q = x_t.rearrange("p (h a w b) -> p a b h w", h=32, a=2, w=32, b=2)
                planes = {}
                for hp in (0, 1):
                    for wp in (0, 1):
                        Hq, Wq = PLANE_HW[(hp, wp)]
                        pq = ppool.tile([128, Hq * Wq], F32R,
                                        tag=f"pl{hp}{wp}{plane_tag_suffix}",
                                        name=f"pl{hp}{wp}_{uid}")
                        p3 = pq.rearrange("p (i j) -> p i j", j=Wq)
                        i0, j0 = 1 - hp, 1 - wp
                        if hp == 0:
                            nc.vector.memset(p3[:, 0:1, :].bitcast(F32), 1.0)
                        if wp == 0:
                            nc.vector.memset(p3[:, :, 0:1].bitcast(F32), 1.0)
                        if no_exp:
                            # timing ablation: keep the x DMA alive via a tiny
                            # DVE consumer into a scratch tile
                            scr = opool.tile([128, 1], F32, tag="scr",
                                             name=f"scr_{uid}_{hp}{wp}")
                            nc.vector.tensor_copy(
                                scr[:, 0:1],
                                x_q[:, 1 - hp, 1 - wp][:, 0:1, 0:1])
                        else:
                            nc.scalar.activation(p3[:, i0:i0 + 32, j0:j0 + 32],
                                                 x_q[:, 1 - hp, 1 - wp], Exp)
                        planes[(hp, wp)] = p3
                return planes

            def gemm_log_store_v4(pair, uid, planes):
                # 4-bank PSUM tiles [128, 2048] per sample: both instance
                # halves and both spatial halves accumulate side by side;
                # one Ln + one (strided) output DMA per sample.
                ps = {}
                for s in (0, 1):
                    ps[s] = pspool.tile([128, 2048], F32, tag=f"ps{s}",
                                        name=f"ps{s}_{uid}")
                for t in range(9):
                    fh, fw = divmod(t, 3)
                    hp, wp = fh & 1, fw & 1
                    io, jo = fh >> 1, fw >> 1
                    p3 = planes[(hp, wp)]
                    for ih in (0, 1):
                        for sh in (0, 1):
                            for s in (0, 1):
                                rhs = p3[64 * s:64 * (s + 1),
                                         sh * 16 + io: sh * 16 + io + 16,
                                         jo: jo + 32]
                                lhsT = expO4[64 * s:64 * (s + 1), t, ih]
                                nc.tensor.matmul(
                                    ps[s][:, (ih * 2 + sh) * 512:(ih * 2 + sh + 1) * 512],
                                    lhsT, rhs,
                                    start=(t == 0), stop=(t == 8),
                                    tile_position=(64 * s, 0))
                for s in (0, 1):
                    ot = opool.tile([128, 2048], F32, tag="out",
                                    name=f"o_{uid}_{s}")
                    nc.scalar.activation(ot[:], ps[s][:], Ln, scale=1.0 / KTOT)
                    # dst: [p, ih, n] where i = ih*128+p, n contiguous 1024
                    dst = y_d[2 * pair + s].rearrange(
                        "(ih p) oh ow -> p ih (oh ow)", ih=2)
                    nc.sync.dma_start(
                        dst[:, :, :],
                        ot.rearrange("p (ih n) -> p ih n", ih=2)[:, :, :])

            def gemm_log_store(pair, uid, planes, dve_ln=False):
                # 2-bank PSUM tiles [128, 1024] per (sample, instance-half):
                # both spatial halves accumulate side by side, one Ln + one
                # contiguous output DMA each.
                for ih in (0, 1):
                    ps = {}
                    for s in (0, 1):
                        ps[s] = pspool.tile([128, 1024], F32, tag=f"ps{s}",
                                            name=f"ps{s}_{uid}_{ih}")
                    for t in range(9):
                        fh, fw = divmod(t, 3)
                        hp, wp = fh & 1, fw & 1
                        io, jo = fh >> 1, fw >> 1
                        p3 = planes[(hp, wp)]
                        for sh in (0, 1):
                            for s in (0, 1):
                                rhs = p3[64 * s:64 * (s + 1),
                                         sh * 16 + io: sh * 16 + io + 16,
                                         jo: jo + 32]
                                lhsT = expO4[64 * s:64 * (s + 1), t, ih]
                                nc.tensor.matmul(
                                    ps[s][:, sh * 512:(sh + 1) * 512], lhsT, rhs,
                                    start=(t == 0), stop=(t == 8),
                                    tile_position=(64 * s, 0))
                    for s in (0, 1):
                        ot = opool.tile([128, 1024], F32, tag="out",
                                        name=f"o_{uid}_{ih}_{s}")
                        if dve_ln:
                            nc.vector.tensor_copy(ot[:], ps[s][:])
                        else:
                            nc.scalar.activation(ot[:], ps[s][:], Ln,
                                                 scale=1.0 / KTOT)
                        dst = y_d[2 * pair + s].rearrange("i oh ow -> i (oh ow)")
                        nc.sync.dma_start(dst[ih * 128:(ih + 1) * 128, :], ot[:])

            for rep in range(reps):
                if variant in ("v1", "v4", "tn", "tl", "tnl"):
                    for pair in range(N_PAIRS):
                        uid = rep * N_PAIRS + pair
                        planes = load_and_exp(pair, uid, "",
                                              no_exp=variant in ("tn", "tnl"))
                        if variant == "v4":
                            gemm_log_store_v4(pair, uid, planes)
                        else:
                            gemm_log_store(pair, uid, planes,
                                           dve_ln=variant in ("tl", "tnl"))
                else:
                    all_planes = []
                    for pair in range(N_PAIRS):
                        uid = rep * N_PAIRS + pair
                        all_planes.append(load_and_exp(pair, uid, f"_{pair}"))
                    for pair in range(N_PAIRS):
                        uid = rep * N_PAIRS + pair
                        gemm_log_store(pair, uid, all_planes[pair])

    nc.compile()
    return nc


def _prep_offsets(offsets: np.ndarray) -> np.ndarray:
    # (1, 256, 64, 3, 3) -> [64, (t, ih, m)] with t = fh*3+fw
    o = offsets.reshape(2, 128, 64, 9)  # (ih, m, c, t)
    return np.ascontiguousarray(o.transpose(2, 3, 0, 1)).reshape(64, 9 * 2 * 128)


def kernel(x: np.ndarray, offsets: np.ndarray) -> np.ndarray:
    from concourse.bass_utils import run_bass_kernel_spmd

    global _compiled
    if _compiled is None:
        _compiled = build_nc()
    nc = _compiled

    offs_t = _prep_offsets(np.asarray(offsets, dtype=np.float32))
    x = np.asarray(x, dtype=np.float32)
    in_maps = [
        {"x": np.ascontiguousarray(x[c * B_CORE:(c + 1) * B_CORE]), "offs": offs_t}
        for c in range(N_CORES)
    ]
    res = run_bass_kernel_spmd(nc, in_maps, list(range(N_CORES)))
    return np.concatenate([res.results[c]["y"] for c in range(N_CORES)], axis=0)

